# revision 3
# baseline (speedup 1.0000x reference)
"""Trainium2 Bass kernel for BQuantConv1d (binary-quantized linear layer).

Computation: out[t, f] = sum_x x[t, x] * W[f, x] + bias[f]
  where W[f, x] = sum_b scale[f, b] * (2*bit(binary[f, b, x//8], x%8) - 1)

Sharding across 8 NeuronCores: hybrid 2-way data-parallel over tokens
(8192 -> 2 x 4096) x 4-way tensor-parallel over output features
(4096 -> 4 x 1024). Each core:
  - builds only HALF its W shard (the pair partner core with the other
    token half builds the other half; halves are exchanged f-tile by f-tile
    with small 2-core AllGathers overlapped with the build),
  - builds W entirely OFF the TensorEngine: DVE unpacks packed bytes to
    {0,1} u8 planes (fused shift+and per bit position, strided out), ScalarE
    applies the per-(f,b) scales during the u8->bf16 cast (w_b = plane*2s_b
    with -sum_b s_b folded into plane 0), and a serial in-slot DVE add chain
    reduces the 8 planes; freeing W-build PSUM lets the main matmul run on
    4+4 double-buffered banks with no pipeline gaps,
  - DMA-transposes the resulting W[f, x] to WT[x, f],
  - streams x tiles with a casting DMA (f32 -> bf16), DMA-transposes them,
    and runs the main matmul out = xT.T @ WT accumulating over the 4096-dim
    contraction in PSUM,
  - adds broadcast bias on PSUM evacuation and DMAs the result out.
"""

from contextlib import ExitStack

import numpy as np

P = 128
BITS = 8
NX = 4096
NB = NX // 8           # packed bytes per feature
NF = 4096
NTOK = 8192            # 4 * 2048
TSHARD = 2             # data-parallel ways (tokens)
FSHARD = 4             # tensor-parallel ways (features)
T_LOC = NTOK // TSHARD     # 4096
NF_LOC = NF // FSHARD      # 1024

MASKV = np.array([128, 64, 32, 16, 8, 4, 2, 1], dtype=np.uint8)

# quarters of the W build (PSUM footprint = NX/WHALVES * 4B per partition)
WHALVES = 2
# cast the unpacked {0,1} u8 planes to bf16 via SWDGE casting DMA instead of
# the ScalarE copy (frees ACT; costs SB->SB DMA bandwidth)
CAST_VIA_DMA = False
# each core of a (token-group) pair builds half of the pair's shared W and
# the halves are exchanged with a 2-core AllGather


def bass_body(ctx: ExitStack, tc, outs, ins, t_loc=T_LOC, nf_loc=NF_LOC,
              replica_groups=None, fake_cc=False, skip_a=False, skip_b=False):
    import concourse.bass as bass  # noqa: F401
    from concourse import mybir
    from concourse.masks import make_identity

    nc = tc.nc
    dt = mybir.dt
    NFT = nf_loc // P          # f-tiles
    NTT = t_loc // P           # t-tiles
    NKC = NX // P              # 32 contraction chunks
    XH = NX // WHALVES         # x columns per W-build half
    BH = NB // WHALVES         # bytes per W-build half
    NWC = XH // 512            # psum-bank chunks per half

    const = ctx.enter_context(tc.tile_pool(name="const", bufs=1))
    wpool = ctx.enter_context(tc.tile_pool(name="wpool", bufs=1))
    wtmp = ctx.enter_context(tc.tile_pool(name="wtmp", bufs=1))
    wmp = ctx.enter_context(tc.tile_pool(name="wmp", bufs=1))
    wsc = ctx.enter_context(tc.tile_pool(name="wsc", bufs=2))
    bitp = ctx.enter_context(tc.tile_pool(name="bitp", bufs=2))
    splp = ctx.enter_context(tc.tile_pool(name="splp", bufs=2))
    xpool = ctx.enter_context(tc.tile_pool(name="xpool", bufs=2))
    xrp = ctx.enter_context(tc.tile_pool(name="xrp", bufs=1))
    opool = ctx.enter_context(tc.tile_pool(name="opool", bufs=1))
    pso0 = ctx.enter_context(tc.tile_pool(name="pso0", bufs=4, space="PSUM"))
    pso1 = ctx.enter_context(tc.tile_pool(name="pso1", bufs=4, space="PSUM"))
    psos = [pso0, pso1]

    # bias -> [128, nf_loc] broadcast (staged via xrp so the slot recycles)
    bias_row = xrp.tile([1, nf_loc], dt.float32, tag="xr", name="bias_row")
    nc.sync.dma_start(bias_row[:], ins["bias_loc"][:].rearrange("(o f) -> o f", o=1))
    bias_bc = const.tile([P, nf_loc], dt.float32)
    nc.gpsimd.partition_broadcast(bias_bc[:], bias_row[:])

    # WT [x-part, kchunk, f] bf16, one tile per 512-wide f-half so matmuls on
    # half 0 can start while half 1 is still being built
    FH = nf_loc // 2
    NFH = 2
    # f-tile-major so each gathered f-tile lands with one contiguous DMA
    wTh = [
        wpool.tile([P, FH // P, NKC, P], dt.bfloat16, name=f"wT{h}", tag=f"wT{h}")
        for h in range(NFH)
    ]
    FT_PER_H = FH // P
    NFT_OWN = NFT // 2         # f-tiles this core builds itself

    cc_in = [
        nc.dram_tensor(f"cc_in{i}", [P, NKC, P], dt.bfloat16).ap()
        for i in range(NFT_OWN)
    ]
    cc_out = [
        nc.dram_tensor(f"cc_out{i}", [NFH, P, NKC, P], dt.bfloat16).ap()
        for i in range(NFT_OWN)
    ]

    def build_ft(ft):
        """Unpack + scale-contract + transpose one 128-feature tile of W."""
        vt = wtmp.tile([P, BITS, NB], dt.uint8, tag="vt", name="vt")
        nc.sync.dma_start(
            vt[:], ins["binary_own"][:].rearrange("(a p) b j -> a p b j", p=P)[ft]
        )
        st = wsc.tile([P, BITS], dt.float32, tag="st", name="st")
        nc.sync.dma_start(
            st[:], ins["scale_own"][:].rearrange("(a p) b -> a p b", p=P)[ft]
        )
        nssum = wsc.tile([P, 1], dt.float32, tag="nssum", name="nssum")
        nc.vector.tensor_reduce(
            out=nssum[:], in_=st[:], axis=mybir.AxisListType.X,
            op=mybir.AluOpType.add, negate=True,
        )
        st2 = wsc.tile([P, BITS], dt.float32, tag="st2", name="st2")
        nc.vector.tensor_scalar_mul(st2[:], st[:], 2.0)

        # W build off the PE: unpack u8 planes (DVE), scaled-cast to bf16 on
        # ACT (w_b = plane*2s_b, -sum_b s_b folded into plane 0), then a
        # serial in-slot add chain split DVE/Pool reduces the 8 planes.
        # Emission: both halves' unpack+cast first (double-buffered spl), so
        # neither DVE nor ACT ever waits on the chains.
        spls = []
        for h in range(WHALVES):
            b8a = bitp.tile([P, BITS, XH], dt.uint8, tag="bits8", name="b8a")
            for p in range(8):
                # bit_p of all 8 planes in one op, strided out at [..., p::8]
                nc.vector.tensor_scalar(
                    out=b8a[:].rearrange("q b (j i) -> q b j i", i=8)[:, :, :, p],
                    in0=vt[:, :, h * BH : (h + 1) * BH],
                    scalar1=int(7 - p),
                    scalar2=int(1),
                    op0=mybir.AluOpType.logical_shift_right,
                    op1=mybir.AluOpType.bitwise_and,
                )
            spl = splp.tile([P, BITS, XH], dt.bfloat16, tag="spl", name="spl")
            for b in range(BITS):
                nc.scalar.activation(
                    out=spl[:, b, :],
                    in_=b8a[:, b, :],
                    func=mybir.ActivationFunctionType.Identity,
                    bias=nssum[:, 0:1] if b == 0 else 0.0,
                    scale=st2[:, b : b + 1],
                )
            spls.append(spl)
        wtx = wmp.tile([P, NKC, P], dt.bfloat16, tag="wtx", name="wtx")
        for h in range(WHALVES):
            with nc.allow_low_precision("bf16 W accumulation (tolerance 2e-2)"):
                S = [spls[h][:, b, :] for b in range(BITS)]
                nc.vector.tensor_tensor(
                    out=S[0], in0=S[0], in1=S[1], op=mybir.AluOpType.add
                )
                for b in range(2, BITS):
                    nc.vector.tensor_tensor(
                        out=S[b - 1], in0=S[b - 2], in1=S[b], op=mybir.AluOpType.add
                    )
            # transpose this half directly (no wm staging tile)
            nc.sync.dma_start(
                wtx[:, h * (NKC // 2) : (h + 1) * (NKC // 2), :],
                S[BITS - 2],
                transpose=True,
            )
        nc.sync.dma_start(cc_in[ft][:], wtx[:])

    def xload(ti):
        xr = xrp.tile([P, NX], dt.bfloat16, tag="xr", name="xr")
        nc.gpsimd.dma_start(
            xr[:], ins["x_loc"][:].rearrange("(a p) x -> a p x", p=P)[ti]
        )
        xT = xpool.tile([P, NKC, P], dt.bfloat16, tag="xT", name="xT")
        nc.sync.dma_start(xT[:], xr[:], transpose=True)
        return xT

    def mm_evac(ti, fhs, xT):
        opss = {
            fh: psos[fh].tile([P, FH], dt.float32, tag=f"ops{fh}", name="ops")
            for fh in fhs
        }
        # k outer / fh inner so both f-halves share each LDWEIGHTS of xT[:,k,:]
        for k in range(NKC):
            for fh in fhs:
                nc.tensor.matmul(
                    opss[fh][:],
                    lhsT=xT[:, k, :],
                    rhs=wTh[fh][:, :, k, :],
                    start=(k == 0),
                    stop=(k == NKC - 1),
                )
        for fh in fhs:
            out_sb = opool.tile([P, FH], dt.float32, tag=f"out{fh}", name="out_sb")
            nc.vector.tensor_tensor(
                out=out_sb[:], in0=opss[fh][:],
                in1=bias_bc[:, fh * FH : (fh + 1) * FH],
                op=mybir.AluOpType.add,
            )
            nc.sync.dma_start(
                outs["out_loc"][:].rearrange("(a p) f -> a p f", p=P)[
                    ti, :, fh * FH : (fh + 1) * FH
                ],
                out_sb[:],
            )

    # ---- schedule: build own W half f-tile by f-tile; exchange each tile
    # with the pair partner via a small 2-core AllGather as soon as it's
    # ready (rank g contributes f-half g), overlapping transfer with build.
    if skip_a:
        for h in range(NFH):
            nc.gpsimd.memset(wTh[h][:], 0.0)
    for ft in range(NFT_OWN if not skip_a else 0):
        build_ft(ft)
        if fake_cc:
            # timing-only stand-in for the collective (TimelineSim has no
            # collectives): same bytes moved dram->dram
            for h in range(NFH):
                nc.sync.dma_start(cc_out[ft][h], cc_in[ft][:])
        else:
            nc.gpsimd.collective_compute(
                "AllGather",
                mybir.AluOpType.bypass,
                replica_groups=replica_groups,
                ins=[cc_in[ft][:]],
                outs=[cc_out[ft][:]],
            )
        for h in range(NFH):
            nc.sync.dma_start(wTh[h][:, ft, :, :], cc_out[ft][h])

    for ti in range(NTT if not skip_b else 0):
        xT = xload(ti)
        mm_evac(ti, list(range(NFH)), xT)


def build_nc(t_loc=T_LOC, nf_loc=NF_LOC, fake_cc=False):
    from concourse import bacc, mybir
    import concourse.tile as tile

    dt = mybir.dt
    n_cores = TSHARD * FSHARD
    nc = bacc.Bacc("TRN2", target_bir_lowering=False, debug=False,
                   num_devices=n_cores)
    ins = {
        "x_loc": nc.dram_tensor("x_loc", [t_loc, NX], dt.float32, kind="ExternalInput").ap(),
        "binary_own": nc.dram_tensor("binary_own", [nf_loc // 2, BITS, NB], dt.uint8, kind="ExternalInput").ap(),
        "scale_own": nc.dram_tensor("scale_own", [nf_loc // 2, BITS], dt.float32, kind="ExternalInput").ap(),
        "bias_loc": nc.dram_tensor("bias_loc", [nf_loc], dt.float32, kind="ExternalInput").ap(),
    }
    outs = {
        "out_loc": nc.dram_tensor("out_loc", [t_loc, nf_loc], dt.float32, kind="ExternalOutput").ap(),
    }
    groups = [[2 * c, 2 * c + 1] for c in range(FSHARD)]
    with tile.TileContext(nc) as tc:
        with ExitStack() as ctx:
            bass_body(ctx, tc, outs, ins, t_loc=t_loc, nf_loc=nf_loc,
                      replica_groups=groups, fake_cc=fake_cc)
    nc.compile()
    return nc


def make_in_maps(x, scale, bias, binary):
    """Shard full inputs into the 8 per-core input maps."""
    xf = np.ascontiguousarray(np.asarray(x, dtype=np.float32).reshape(NTOK, NX))
    b8 = np.ascontiguousarray(
        np.asarray(binary).reshape(NF, BITS, NB).astype(np.uint8)
    )
    s2 = np.ascontiguousarray(np.asarray(scale, dtype=np.float32).reshape(NF, BITS))
    bb = np.ascontiguousarray(np.asarray(bias, dtype=np.float32))
    in_maps = []
    H = NF_LOC // 2
    for core in range(TSHARD * FSHARD):
        c, g = divmod(core, TSHARD)
        f0 = c * NF_LOC + g * H
        in_maps.append(
            {
                "x_loc": xf[g * T_LOC : (g + 1) * T_LOC],
                "binary_own": b8[f0 : f0 + H],
                "scale_own": s2[f0 : f0 + H],
                "bias_loc": bb[c * NF_LOC : (c + 1) * NF_LOC],
            }
        )
    return in_maps


def assemble_output(results, out_shape=(4, 2048, NF)):
    out = np.empty((NTOK, NF), dtype=np.float32)
    for core in range(TSHARD * FSHARD):
        c, g = divmod(core, TSHARD)
        out[g * T_LOC : (g + 1) * T_LOC, c * NF_LOC : (c + 1) * NF_LOC] = results[
            core
        ]["out_loc"]
    return out.reshape(out_shape)


_NC_CACHE = {}


def _get_nc():
    if "nc" not in _NC_CACHE:
        _NC_CACHE["nc"] = build_nc()
    return _NC_CACHE["nc"]


def run_on_hw(x, scale, bias, binary, trace=False, **kwargs):
    from concourse.bass_utils import run_bass_kernel_spmd

    nc = _get_nc()
    in_maps = make_in_maps(x, scale, bias, binary)
    res = run_bass_kernel_spmd(
        nc, in_maps, core_ids=list(range(TSHARD * FSHARD)), trace=trace, **kwargs
    )
    return res


def kernel(x, scale, bias, binary):
    res = run_on_hw(x, scale, bias, binary, trace=False)
    return assemble_output(res.results, out_shape=np.asarray(x).shape[:-1] + (NF,))


if __name__ == "__main__":
    rng = np.random.default_rng(0)
    x = rng.standard_normal((4, 2048, NX), dtype=np.float32)
    scale = rng.random((NF, 1, BITS), dtype=np.float32)
    bias = rng.standard_normal(NF).astype(np.float32)
    binary = rng.integers(0, 256, size=(NF, BITS, NB, 1), dtype=np.int32).astype(np.int8)
    out = kernel(x, scale, bias, binary)
    print(out.shape, out.dtype)

